# revision 44
# baseline (speedup 1.0000x reference)
"""Causal attention kernel for Trainium2 (Bass/Tile), 8-core data-parallel.

Problem: x[32,1024,512] f32, W[512,1536] f32.
  kqv = x @ W; k,q,v = split(kqv); S = q k^T / sqrt(512) (causal);
  out = softmax(S) @ v.

Distribution: batch-parallel, 4 batches per core, weights replicated.

Per-core algorithm (per batch):
  - kT/qT ([C,T], C on partitions) via fp8 DoubleRow matmuls: host
    pre-interleaves x and W in contraction pairs ((p,j) <-> c=2p+j per
    128-pair chunk) and pre-permutes W columns so the kT/qT PSUM output
    partitions land directly in the pair-interleaved layout the scores
    matmul needs. W is pre-scaled by 32 to clear the fp8 subnormal range.
  - v ([T,C]) in float32r (full fp32 data, fast PE streaming mode).
  - Scores computed TRANSPOSED: ST[s,t] = k q^T via fp8 DoubleRow, so
    softmax normalization can be deferred: P^T = exp(ST*scale) (no
    max-subtraction: scores ~N(0,0.2), exp is safe), causal handled by
    skipping upper-triangle 128-blocks + one triangular mask multiply on
    the diagonal block.
  - out_raw = P^T v in float32r single-bank [128,512] matmuls; row-sums
    via tiny ones-column matmuls into a shared [128,8] PSUM tile (sharing
    each PV matmul's stationary); out = out_raw * (1/rowsum).
  - Matmul emission order shares stationary (Ldweights) between
    consecutive matmuls wherever possible: G pairs both x-halves under
    one m8 slice, V runs (x8,w8v),(x8,wr8v),(xr8,w8v), ST hoists the
    x8 slice over both t-chunks.
"""

import sys

sys.path.insert(0, "/opt/trn_rl_repo")

import numpy as np

import concourse.mybir as mybir
import concourse.tile as tile
from concourse import bacc
from concourse.bass_utils import run_bass_kernel_spmd

B, T, C = 32, 1024, 512
N_CORES = 8
BPC = B // N_CORES  # 4 batches per core
P = 128
NT = T // P  # 8 row tiles of T
NU = C // (2 * P)  # 2 pair-chunks of C (128 pairs each)
F32 = mybir.dt.float32
F32R = mybir.dt.float32r
FP8 = mybir.dt.float8e4
FP8E5 = mybir.dt.float8e5
EXP = mybir.ActivationFunctionType.Exp
DR = mybir.MatmulPerfMode.DoubleRow

W_SCALE = 32.0  # pre-scale for Wv in fp8 (clears subnormals)
M_SCALE = 64.0  # pre-scale for M = Wk Wq^T in fp8
SCORE_SCALE = float(C) ** -0.5 / M_SCALE

NP_FP8 = mybir.dt.np(FP8)
NP_FP8E5 = mybir.dt.np(FP8E5)

_CACHE = {}

# emission variants (model-bisect knobs)
G_PAIR = True  # G: share m8 stationary across both x-halves (2 psums open)
ST_HOIST = True  # ST: share x8 stationary across both t-chunks (2 psums open)
G8_SPLIT = False  # G: drain half-psums on ACT+DVE instead of ACT only
V_ALT = False  # V: alternate psum-drain engine between DVE and ACT
PV7_EARLY = False  # last batch: open PV(7) psum chain one ST step early
MASK_POOL = False  # diag mask multiply on Pool (gpsimd) vs DVE


def build_bass(repeats=1):
    nc = bacc.Bacc(None, target_bir_lowering=False)
    # x8: pair-interleaved fp8 x^T: [BPC, u, p, j, t] <-> x[b, t, 256u+2p+j]
    x8_d = nc.declare_dram_parameter("x8", [BPC, NU, P, 2, T], FP8, isOutput=False)
    # xr8: e5m2 residual x - fp8(x), same pair-interleaved layout — V is
    # computed residual-compensated in fp8 DoubleRow:
    #   32 v = x8·(32Wv)8 + xr·(32Wv)8 + x8·(32Wv − (32Wv)8)
    xr8_d = nc.declare_dram_parameter("xr8", [BPC, NU, P, 2, T], FP8E5, isOutput=False)
    # m8: M^T where M = Wk Wq^T (precomputed host-side so scores need only
    # ONE on-chip projection G = M x^T instead of kT and qT):
    # pair-interleaved rows (d), column-permuted (c' blocks (u',j')), x64
    m8_d = nc.declare_dram_parameter("m8", [NU, P, 2, C], FP8, isOutput=False)
    # w8v: fp8(32 Wv), pair-interleaved rows; wr8v: e5m2 residual of it
    w8v_d = nc.declare_dram_parameter("w8v", [NU, P, 2, C], FP8, isOutput=False)
    wr8v_d = nc.declare_dram_parameter("wr8v", [NU, P, 2, C], FP8E5, isOutput=False)
    # triangular keep-mask for diagonal blocks (upper-tri incl diag), f32
    mask_d = nc.declare_dram_parameter("mask", [P, P], F32R, isOutput=False)
    # [32,32] per partition: rowsum matmul rhs (width 2: f32r matmuls
    # need free >= 2); 32 matches the 32v scale of the compensated V so
    # normalization cancels it for free
    ones_d = nc.declare_dram_parameter("ones", [P, 2], F32R, isOutput=False)
    out_d = nc.declare_dram_parameter("out", [BPC, T, C], F32, isOutput=True)

    with tile.TileContext(nc) as tc:
        with (
            tc.tile_pool(name="const", bufs=1) as constp,
            tc.tile_pool(name="x8", bufs=2) as x8p,
            tc.tile_pool(name="xt", bufs=2) as xtp,
            tc.tile_pool(name="kq", bufs=2) as kqp,
            tc.tile_pool(name="v", bufs=2) as vp,
            tc.tile_pool(name="pt", bufs=3) as ptp,
            tc.tile_pool(name="osb", bufs=4) as osbp,
            tc.tile_pool(name="rec", bufs=4) as recp,
            tc.tile_pool(name="ps", bufs=2, space="PSUM") as psp,
            tc.tile_pool(name="psv", bufs=2, space="PSUM") as psvp,
            tc.tile_pool(name="pso", bufs=2, space="PSUM") as psop,
            tc.tile_pool(name="psr", bufs=2, space="PSUM") as psrp,
        ):
            # m8 first, then batch-0 x8 is emitted before the remaining
            # constants so the first G matmuls can start ~3us earlier.
            m8t = []
            for u in range(NU):
                t_ = constp.tile([P, 2, C], FP8, tag=f"m8{u}")
                if u == 0:
                    nc.sync.dma_start(t_[:], m8_d[u])
                m8t.append(t_)
            w8vt = [constp.tile([P, 2, C], FP8, tag=f"w8v{u}", name=f"w8v_{u}")
                    for u in range(NU)]
            wr8vt = [constp.tile([P, 2, C], FP8E5, tag=f"wr8v{u}", name=f"wr8v_{u}")
                     for u in range(NU)]
            maskt = constp.tile([P, P], F32R, tag="mask")
            onest = constp.tile([P, 2], F32R, tag="ones")

            def fetch_x8(li_, b_, cold=False):
                # Cold start: x8 rides the ACT DGE queue so its transfer
                # overlaps the SP queue's m8/consts instead of serializing
                # behind them, split into h-halves so the h0 pieces the
                # first G chains need land ~0.5us sooner.
                x8s_ = [
                    x8p.tile([P, 2, T], FP8, tag=f"x8{u}", name=f"x8_{li_}_{u}")
                    for u in range(NU)
                ]
                for u in range(NU):
                    (nc.scalar if cold else nc.sync).dma_start(x8s_[u][:], x8_d[b_, u])
                return x8s_

            def fetch_xr8(li_, b_):
                xr8s_ = []
                for u in range(NU):
                    t_ = xtp.tile([P, 2, T], FP8E5, tag=f"xr8{u}", name=f"xr8_{li_}_{u}")
                    nc.sync.dma_start(t_[:], xr8_d[b_, u])
                    xr8s_.append(t_)
                return xr8s_

            seq = [b for _ in range(repeats) for b in range(BPC)]
            n_iters = len(seq)
            pend_x8 = pend_xr8 = None
            for li, b in enumerate(seq):
                last_batch = li == n_iters - 1
                prefetch = li + 1 < n_iters
                if li == 0:
                    # HWDGE gen is serial across queues (~630ns each): emit
                    # the first G chain's inputs in need-order.
                    x8s = [
                        x8p.tile([P, 2, T], FP8, tag=f"x8{u}", name=f"x8_0_{u}")
                        for u in range(NU)
                    ]
                    nc.scalar.dma_start(x8s[0][:], x8_d[b, 0])
                    nc.sync.dma_start(m8t[1][:], m8_d[1])
                    nc.scalar.dma_start(x8s[1][:], x8_d[b, 1])
                    xr8s = fetch_xr8(0, b)
                    for u in range(NU):
                        nc.sync.dma_start(w8vt[u][:], w8v_d[u])
                        nc.sync.dma_start(wr8vt[u][:], wr8v_d[u])
                    nc.sync.dma_start(maskt[:], mask_d[:])
                    nc.sync.dma_start(onest[:], ones_d[:])
                else:
                    x8s = fetch_x8(li, b)
                    xr8s = fetch_xr8(li, b)

                # G = M x^T via fp8 DoubleRow. Output block (u',j') covers
                # rows c' = 256u' + 2p + j' of G, written pair-interleaved
                # into g8t[u'][:, j', :] so ST can contract x8 against it.
                g8t = [
                    kqp.tile([P, 2, T], FP8, tag=f"g8{u}", name=f"g8_{b}_{u}")
                    for u in range(NU)
                ]
                # Projections, interleaved: G groups are copy-bound (fast
                # fp8-DR matmuls, ACT PSUM drains) while V groups are
                # PE+DVE — mixing them keeps PE, ACT and DVE all busy.
                vs = [None] * NT

                def emit_v(tj):
                    # V group: residual-compensated fp8 DR — psum accumulates
                    # 32v = x8·w8v + x8·wrv + xr·w8v, ordered so x8 serves two
                    # consecutive matmuls per u (one stationary load).
                    ps = psvp.tile([P, 512], F32, tag="psv", name=f"psv{b}_{tj}")
                    terms = []
                    for u in range(NU):
                        xs = x8s[u][:, :, tj * P : (tj + 1) * P]
                        xrs = xr8s[u][:, :, tj * P : (tj + 1) * P]
                        terms += [(xs, w8vt[u]), (xs, wr8vt[u]), (xrs, w8vt[u])]
                    for ti, (lhs, rhs) in enumerate(terms):
                        nc.tensor.matmul(
                            ps[:], lhs, rhs[:],
                            start=(ti == 0),
                            stop=(ti == len(terms) - 1),
                            perf_mode=DR,
                        )
                    sb = vp.tile([P, C], F32R, tag=f"v{tj}", name=f"v_{b}_{tj}")
                    # alternate psum-drain engine to balance ACT/DVE
                    if V_ALT and tj % 2 == 1:
                        nc.scalar.copy(sb[:], ps[:])
                    else:
                        nc.vector.tensor_copy(sb[:], ps[:])
                    vs[tj] = sb

                # G: both t-halves under one m8 stationary slice (2 psum
                # tiles per c'-pair, u-accumulated).
                if G_PAIR:
                    for p4 in range(4):
                        up, jp = p4 // 2, p4 % 2
                        psh = [psp.tile([P, 512], F32, tag="ps", name=f"psg{b}_{p4}_{h}")
                               for h in range(2)]
                        for u in range(NU):
                            lhs = m8t[u][:, :, p4 * P : (p4 + 1) * P]
                            for h in range(2):
                                nc.tensor.matmul(
                                    psh[h][:],
                                    lhs,
                                    x8s[u][:, :, h * 512 : (h + 1) * 512],
                                    start=(u == 0),
                                    stop=(u == NU - 1),
                                    perf_mode=DR,
                                )
                        nc.scalar.copy(g8t[up][:, jp, 0:512], psh[0][:])
                        nc.scalar.copy(g8t[up][:, jp, 512:1024], psh[1][:])
                        if li > 0:
                            emit_v(2 * p4)
                            emit_v(2 * p4 + 1)
                else:
                    for bi in range(8):
                        up, jp, h = (bi // 2) // 2, (bi // 2) % 2, bi % 2
                        ps = psp.tile([P, 512], F32, tag="ps", name=f"psg{b}_{bi}")
                        for u in range(NU):
                            nc.tensor.matmul(
                                ps[:],
                                m8t[u][:, :, (2 * up + jp) * P : (2 * up + jp + 1) * P],
                                x8s[u][:, :, h * 512 : (h + 1) * 512],
                                start=(u == 0),
                                stop=(u == NU - 1),
                                perf_mode=DR,
                            )
                        nc.scalar.copy(g8t[up][:, jp, h * 512 : (h + 1) * 512], ps[:])
                        if li > 0:
                            emit_v(bi)

                # out[tj] = (sum_{i<=tj} PT_i^T v_i) / rowsum; rowsum via a
                # tiny ones-matmul per (i,tj) sharing the PV stationary,
                # accumulated into column tj of a per-batch [P,NT] psum.
                # Emitted interleaved with the ST/exp loop below: PE runs
                # PV(si-1) while ACT exps ST(si).
                pts = []

                def emit_pv(tj, drain_split=False):
                    psr_t = psrp.tile([P, 2], F32, tag="psr", name=f"psr{b}_{tj}")
                    ps_o = psop.tile([P, C], F32, tag="pso", name=f"pso{b}_{tj}")
                    for i in range(tj + 1):
                        st, sp = (i == 0), (i == tj)
                        lhs = pts[i][:, tj * P : (tj + 1) * P]
                        # rowsum mm first: its stop-sem pipeline latency (173ns)
                        # hides under the big PV matmul that follows.
                        nc.tensor.matmul(psr_t[:], lhs, onest[:], start=st, stop=sp)
                        nc.tensor.matmul(ps_o[:], lhs, vs[i][:], start=st, stop=sp)
                    finish_pv(tj, ps_o, psr_t, drain_split)

                def finish_pv(tj, ps_o, psr_t, drain_split=False):
                    rec = recp.tile([P, 1], F32, tag="rec", name=f"rec{b}_{tj}")
                    nc.vector.reciprocal(rec[:], psr_t[:, :1])
                    osb = osbp.tile([P, C], F32, tag="osb", name=f"osb{b}_{tj}")
                    lo = tj * P
                    if drain_split:
                        # tail drain: halve the normalize across DVE+ACT and
                        # the out-DMA across the SP+ACT DGE queues so the
                        # final output drain pipelines instead of serializing
                        H = C // 2
                        nc.vector.tensor_scalar_mul(osb[:, :H], ps_o[:, :H], rec[:])
                        nc.sync.dma_start(out_d[b, lo : lo + P, :H], osb[:, :H])
                        nc.scalar.activation(
                            osb[:, H:], ps_o[:, H:],
                            mybir.ActivationFunctionType.Copy, scale=rec[:],
                        )
                        nc.scalar.dma_start(out_d[b, lo : lo + P, H:], osb[:, H:])
                    else:
                        nc.vector.tensor_scalar_mul(osb[:], ps_o[:], rec[:])
                        nc.sync.dma_start(out_d[b, lo : lo + P, :], osb[:])

                # the last batch opens PV(7)'s psum chain early (terms
                # i=0..6 are ready one ST step before the final exp), so
                # after the last exp only one matmul + norm + DMA remain.
                pv7 = {}

                def open_pv7():
                    # pso7 borrows a V-psum buf (the V chains all drained
                    # during the G phase), so the rolling PV chains keep both
                    # pso bufs and don't serialize behind this open chain.
                    tj = NT - 1
                    pv7["psr"] = psrp.tile([P, 2], F32, tag="psr", name=f"psr7o{b}")
                    pv7["pso"] = psvp.tile([P, C], F32, tag="psv", name=f"pso7o{b}")
                    for i in range(NT - 1):
                        st = (i == 0)
                        lhs = pts[i][:, tj * P : (tj + 1) * P]
                        nc.tensor.matmul(pv7["psr"][:], lhs, onest[:], start=st, stop=False)
                        nc.tensor.matmul(pv7["pso"][:], lhs, vs[i][:], start=st, stop=False)

                def close_pv7():
                    tj = NT - 1
                    lhs = pts[tj][:, tj * P : (tj + 1) * P]
                    nc.tensor.matmul(pv7["psr"][:], lhs, onest[:], start=False, stop=True)
                    nc.tensor.matmul(pv7["pso"][:], lhs, vs[tj][:], start=False, stop=True)
                    finish_pv(tj, pv7["pso"], pv7["psr"])

                # P^T tiles: PT[s,t] = exp(scale' * (32k)·(32q)), causal.
                # The x8 stationary slice is hoisted over both t-chunks.
                for si in range(NT):
                    lo = si * P
                    pt_t = ptp.tile([P, T], F32R, tag=f"pt{si}")
                    w_all = T - lo
                    if w_all > 512:
                        half = (w_all // 2 + 127) // 128 * 128
                        chunks = [(lo, lo + half), (lo + half, T)]
                    else:
                        chunks = [(lo, T)]
                    if ST_HOIST:
                        pss = [psp.tile([P, 512], F32, tag="ps", name=f"psst{b}_{si}_{ci}")
                               for ci in range(len(chunks))]
                        for u in range(NU):
                            lhs = x8s[u][:, :, lo : lo + P]
                            for ci, (t0, t1) in enumerate(chunks):
                                nc.tensor.matmul(
                                    pss[ci][:, : t1 - t0],
                                    lhs,
                                    g8t[u][:, :, t0:t1],
                                    start=(u == 0),
                                    stop=(u == NU - 1),
                                    perf_mode=DR,
                                )
                        for ci, (t0, t1) in enumerate(chunks):
                            nc.scalar.activation(
                                pt_t[:, t0:t1], pss[ci][:, : t1 - t0], EXP,
                                scale=SCORE_SCALE,
                            )
                    else:
                        for t0, t1 in chunks:
                            w_ = t1 - t0
                            ps = psp.tile([P, 512], F32, tag="ps", name=f"psst{b}_{si}_{t0}")
                            for u in range(NU):
                                nc.tensor.matmul(
                                    ps[:, :w_],
                                    x8s[u][:, :, lo : lo + P],
                                    g8t[u][:, :, t0:t1],
                                    start=(u == 0),
                                    stop=(u == NU - 1),
                                    perf_mode=DR,
                                )
                            nc.scalar.activation(
                                pt_t[:, t0:t1], ps[:, :w_], EXP, scale=SCORE_SCALE
                            )
                    (nc.gpsimd if MASK_POOL else nc.vector).tensor_mul(
                        pt_t[:, lo : lo + P], pt_t[:, lo : lo + P], maskt[:]
                    )
                    pts.append(pt_t)
                    if li > 0 and si >= 1:
                        emit_pv(si - 1)
                        if PV7_EARLY and last_batch and si == NT - 2:
                            open_pv7()
                if li == 0:
                    # cold-start batch: V-path inputs (w8v/xr8) arrive last,
                    # so V/PV are emitted after ST to keep PE fed during the
                    # DMA ramp-in.
                    for tj in range(NT):
                        emit_v(tj)
                    for tj in range(NT - 1):
                        emit_pv(tj)
                if PV7_EARLY and last_batch and li > 0:
                    close_pv7()
                else:
                    emit_pv(NT - 1)

    nc.compile()
    return nc


def prep_inputs(x: np.ndarray, W_attn: np.ndarray):
    """Host-side sharding + layout transforms. Returns in_maps for 8 cores."""
    xt = np.ascontiguousarray(np.transpose(x, (0, 2, 1)))  # [B, C, T] f32
    # pair-interleaved fp8 x^T: [B, NU, P, 2, T], plus e5m2 residual
    xp = np.ascontiguousarray(xt.reshape(B, NU, P, 2, T))
    x8 = xp.astype(NP_FP8)
    xr8 = (xp - x8.astype(np.float32)).astype(NP_FP8E5)

    # M = Wk Wq^T precomputed host-side; shipped as M^T (contraction d on
    # rows), pair-interleaved rows, columns c' permuted into (u',j') blocks.
    wk, wq = W_attn[:, :C], W_attn[:, C : 2 * C]
    mt = (wk @ wq.T).T * M_SCALE  # [d, c']
    cols = []
    for up in range(2):
        for jp in range(2):
            cols.append(256 * up + jp + 2 * np.arange(P))
    colperm = np.concatenate(cols)
    m8 = mt[:, colperm].reshape(NU, P, 2, C).astype(NP_FP8)
    # Wv: 32x-scaled fp8 + e5m2 residual, pair-interleaved rows
    wv32 = np.ascontiguousarray(W_attn[:, 2 * C :] * W_SCALE).reshape(NU, P, 2, C)
    w8v = wv32.astype(NP_FP8)
    wr8v = (wv32 - w8v.astype(np.float32)).astype(NP_FP8E5)

    mask = np.triu(np.ones((P, P), dtype=np.float32))
    ones = np.full((P, 2), W_SCALE, dtype=np.float32)

    in_maps = []
    for c in range(N_CORES):
        sl = slice(c * BPC, (c + 1) * BPC)
        in_maps.append(
            {
                "x8": x8[sl],
                "xr8": xr8[sl],
                "m8": m8,
                "w8v": w8v,
                "wr8v": wr8v,
                "mask": mask,
                "ones": ones,
            }
        )
    return in_maps


def kernel(x: np.ndarray, W_attn: np.ndarray) -> np.ndarray:
    x = np.asarray(x, dtype=np.float32)
    W_attn = np.asarray(W_attn, dtype=np.float32)
    if "nc" not in _CACHE:
        _CACHE["nc"] = build_bass()
    nc = _CACHE["nc"]
    in_maps = prep_inputs(x, W_attn)
    res = run_bass_kernel_spmd(nc, in_maps, list(range(N_CORES)))
    out = np.concatenate([res.results[c]["out"] for c in range(N_CORES)], axis=0)
    return out.astype(np.float32)

